# revision 3
# baseline (speedup 1.0000x reference)
"""SkeletalPool Trainium2 kernel, v4: three decoupled DMA rings, DVE-only
compute, bf16 stores, DRAM->DRAM root copies.

Computes out = (x[:, IDX0] + x[:, IDX1]) * 0.5 for the skeletal pooling
map: joint 0 passes through, joints (2i-1, 2i) average into output joint
i (i = 1..15).

  x:   [32, 31, 64, 4096] f32
  out: [32, 16, 64, 4096] bf16 on device, upcast to f32 on host.

Sharding: pure data parallelism over batch - 32 batches / 8 cores = 4
per core, no communication.

Measured hardware facts this design is built on (probe.py):
  - The HBM->SBUF load channel saturates at ~335 GB/s per core no matter
    how many DMA rings issue (1 ring: 333, 2: 342, 3: 301 GB/s). The
    kernel is load-bound: 120 MB of pair loads / ~335 GB/s ~ 358 us.
  - SBUF->HBM stores ride concurrently (baseline sustained 162 GB/s of
    writes on top of 314 GB/s reads), so bf16 stores (30 MB) are free.
  - DRAM->DRAM is slow (~123 GB/s) and interferes with loads, so only
    the 4 root-joint copies (pass-through, no compute) go that way -
    removing their 4 MB from the saturated load channel.
  - SWDGE (Pool) DMAs match HWDGE rings (~340 GB/s) and are the only
    path that can cast f32->bf16 in flight (used for the root copies).

Structure:
  - Work unit = 1 pair: load 2 input joints [128, 2, 2048] (8 KB
    descriptors, full 128-partition engagement - 16 KB descriptors with
    64 partitions measured SLOWER: 209 GB/s), DVE adds into a bf16 sum
    (one rounding, rel err <= 2^-8) and scales by 0.5 in place (exact,
    power of two), then the sum tile stores out. 60 units per rep.
  - Unit g's load and store both issue on ring g % 3 (SP / ACT / Pool);
    the store trails its ring by SLAG=3 indices. No compute runs on the
    ring engines, so rings never stall on the compute chain beyond the
    explicit slot-reuse gates.
  - Semaphore rules learned from the race detector / SWDGE ucode model:
    sems updated by SWDGE must be SWDGE-exclusive and hit exact
    cumulative targets; DMA +16 updates arrive as partial increments so
    a wait at 16n is exact only with one DMA in flight per sem. Slot
    counts are multiples of 3 so each slot sem is owned by exactly one
    ring, and slot reuse is gated through the compute chain (s_mul).
"""

import sys

if "/opt/trn_rl_repo" not in sys.path:
    sys.path.insert(0, "/opt/trn_rl_repo")

import numpy as np

import concourse.bass as bass
import concourse.mybir as mybir
from concourse.bass_utils import run_bass_kernel_spmd

N_CORES = 8
B_FULL = 32
B_SHARD = B_FULL // N_CORES  # 4
J_IN = 31
J_OUT = 16
C = 64
T = 4096
P = 128
TT = (C * T) // P  # 2048
N_PAIRS = 15
N_UNITS = B_SHARD * N_PAIRS  # 60 pair units per rep; roots go DRAM->DRAM
NB_IN = 9  # tin slots (multiple of 3: slot sems ring-exclusive)
NB_SUM = 9  # tsum slots (multiple of 3; store sems ring-exclusive)
SLAG = 3  # store for unit j-3 issued at ring index j (3 | SLAG keeps ring)
N_RINGS = 3

f32 = mybir.dt.float32
bf16 = mybir.dt.bfloat16

_CACHE = {}


def _build_nc(reps: int = 1, out_dt=bf16) -> bass.Bass:
    nc = bass.Bass("TRN2", debug=False, num_devices=N_CORES)
    x = nc.dram_tensor("x", (B_SHARD, J_IN, C, T), f32, kind="ExternalInput")
    out = nc.dram_tensor("out", (B_SHARD, J_OUT, C, T), out_dt, kind="ExternalOutput")
    xp = x.ap().rearrange("b j c (u t) -> b (c u) j t", u=2)  # [4,128,31,2048]
    op = out.ap().rearrange("b j c (u t) -> b (c u) j t", u=2)  # [4,128,16,2048]

    tin = nc.alloc_sbuf_tensor("tin", [P, NB_IN * 2 * TT], f32)
    tsum = nc.alloc_sbuf_tensor("tsum", [P, NB_SUM * TT], out_dt)
    s_load = [nc.alloc_semaphore(f"s_load{i}") for i in range(NB_IN)]
    s_store = [nc.alloc_semaphore(f"s_store{i}") for i in range(NB_SUM)]
    s_root = [nc.alloc_semaphore(f"s_root{i}") for i in range(2)]
    s_add = nc.alloc_semaphore("s_add")
    s_mul = nc.alloc_semaphore("s_mul")

    TOT = reps * N_UNITS
    N_ROOTS = reps * B_SHARD

    def task(g):
        b, k1 = divmod(g % N_UNITS, N_PAIRS)
        return b, k1 + 1  # output joint k; inputs (2k-1, 2k)

    def tin_v(g):
        s = (g % NB_IN) * 2 * TT
        return tin.ap()[:, s : s + 2 * TT].rearrange("p (j t) -> p j t", j=2)

    def tsum_v(g):
        s = (g % NB_SUM) * TT
        return tsum.ap()[:, s : s + TT]

    def issue_load(eng, g):
        b, k = task(g)
        if g >= NB_IN:
            # tin slot free once unit g-NB_IN's add has read it
            eng.wait_ge(s_mul, g - NB_IN + 1)
        j0 = 2 * k - 1
        eng.dma_start(out=tin_v(g), in_=xp[b, :, j0 : j0 + 2, :]).then_inc(
            s_load[g % NB_IN], 16
        )

    def issue_store(eng, g):
        b, k = task(g)
        eng.wait_ge(s_mul, g + 1)
        eng.dma_start(out=op[b, :, k, :], in_=tsum_v(g)).then_inc(
            s_store[g % NB_SUM], 16
        )

    def issue_root(eng, ri):
        # root joint passes through exactly: DRAM->DRAM cast copy (SWDGE),
        # riding the D2D path instead of the saturated HBM->SBUF channel.
        b = ri % B_SHARD
        if ri >= 2:
            eng.wait_ge(s_root[ri % 2], 16 * (ri // 2))
        eng.dma_start(out=op[b, :, 0, :], in_=xp[b, :, 0, :]).then_inc(
            s_root[ri % 2], 16
        )

    # fire root (rep, b) at the first Pool ring index inside that batch's
    # unit range
    root_at = {}
    for ri in range(N_ROOTS):
        j0 = ri * N_PAIRS
        root_at[j0 + ((2 - j0) % N_RINGS)] = ri

    def ring_prog(eng, r):
        for j in range(TOT + SLAG):
            if r == 2 and j in root_at:
                issue_root(eng, root_at[j])
            if j < TOT and j % N_RINGS == r:
                issue_load(eng, j)
            js = j - SLAG
            if js >= 0 and j % N_RINGS == r:
                issue_store(eng, js)

    with nc.Block() as block:

        @block.sync
        def _(sync):
            ring_prog(sync, 0)
            # gate kernel end on all stores and roots; counts are exact
            # because slot reuse serializes same-sem DMAs.
            sync.wait_ge(s_mul, TOT)
            for s in range(NB_SUM):
                sync.wait_ge(s_store[s], 16 * len(range(s, TOT, NB_SUM)))
            for i in range(2):
                sync.wait_ge(s_root[i], 16 * len(range(i, N_ROOTS, 2)))

        @block.scalar
        def _(scalar):
            ring_prog(scalar, 1)

        @block.gpsimd
        def _(gpsimd):
            ring_prog(gpsimd, 2)

        @block.vector
        def _(vector):
            for g in range(TOT):
                vector.wait_ge(s_load[g % NB_IN], 16 * (g // NB_IN + 1))
                if g >= NB_SUM:
                    # tsum slot free once unit g-NB_SUM's store completed
                    vector.wait_ge(s_store[g % NB_SUM], 16 * (g // NB_SUM))
                tv = tin_v(g)
                sv = tsum_v(g)
                # DVE pipelines instructions (queue depth 8), so the RAW
                # add -> mul on the same tile needs an explicit sem edge
                vector.tensor_add(out=sv, in0=tv[:, 0, :], in1=tv[:, 1, :]).then_inc(
                    s_add, 1
                )
                vector.wait_ge(s_add, g + 1)
                # in-place halve: exact (power of two), same-AP in/out
                vector.tensor_scalar_mul(sv, sv, 0.5).then_inc(s_mul, 1)

    return nc


def get_nc() -> bass.Bass:
    if "nc" not in _CACHE:
        _CACHE["nc"] = _build_nc(1)
    return _CACHE["nc"]


def kernel(x: np.ndarray, **run_kwargs):
    x = np.ascontiguousarray(np.asarray(x, dtype=np.float32))
    assert x.shape == (B_FULL, J_IN, C, T), x.shape

    nc = get_nc()
    in_maps = [
        {"x": np.ascontiguousarray(x[i * B_SHARD : (i + 1) * B_SHARD])}
        for i in range(N_CORES)
    ]
    res = run_bass_kernel_spmd(nc, in_maps, core_ids=list(range(N_CORES)), **run_kwargs)
    out = np.concatenate(
        [np.asarray(res.results[i]["out"]) for i in range(N_CORES)], axis=0
    ).astype(np.float32)
    _CACHE["last_results"] = out if False else res
    return out


# revision 4
# speedup vs baseline: 1.2020x; 1.2020x over previous
"""SkeletalPool Trainium2 kernel, v4: three decoupled DMA rings, DVE-only
compute, bf16 stores, DRAM->DRAM root copies.

Computes out = (x[:, IDX0] + x[:, IDX1]) * 0.5 for the skeletal pooling
map: joint 0 passes through, joints (2i-1, 2i) average into output joint
i (i = 1..15).

  x:   [32, 31, 64, 4096] f32
  out: [32, 16, 64, 4096] bf16 on device, upcast to f32 on host.

Sharding: pure data parallelism over batch - 32 batches / 8 cores = 4
per core, no communication.

Measured hardware facts this design is built on (probe.py):
  - The HBM->SBUF load channel saturates at ~335 GB/s per core no matter
    how many DMA rings issue (1 ring: 333, 2: 342, 3: 301 GB/s). The
    kernel is load-bound: 120 MB of pair loads / ~335 GB/s ~ 358 us.
  - SBUF->HBM stores ride concurrently (baseline sustained 162 GB/s of
    writes on top of 314 GB/s reads), so bf16 stores (30 MB) are free.
  - DRAM->DRAM is slow (~123 GB/s) and interferes with loads, so only
    the 4 root-joint copies (pass-through, no compute) go that way -
    removing their 4 MB from the saturated load channel.
  - SWDGE (Pool) DMAs match HWDGE rings (~340 GB/s) and are the only
    path that can cast f32->bf16 in flight (used for the root copies).

Structure:
  - Work unit = 1 pair: load 2 input joints [128, 2, 2048] (8 KB
    descriptors, full 128-partition engagement - 16 KB descriptors with
    64 partitions measured SLOWER: 209 GB/s), DVE adds into a bf16 sum
    (one rounding, rel err <= 2^-8) and scales by 0.5 in place (exact,
    power of two), then the sum tile stores out. 60 units per rep.
  - Unit g's load and store both issue on ring g % 3 (SP / ACT / Pool);
    the store trails its ring by SLAG=3 indices. No compute runs on the
    ring engines, so rings never stall on the compute chain beyond the
    explicit slot-reuse gates.
  - Semaphore rules learned from the race detector / SWDGE ucode model:
    sems updated by SWDGE must be SWDGE-exclusive and hit exact
    cumulative targets; DMA +16 updates arrive as partial increments so
    a wait at 16n is exact only with one DMA in flight per sem. Slot
    counts are multiples of 3 so each slot sem is owned by exactly one
    ring, and slot reuse is gated through the compute chain (s_mul).
"""

import sys

if "/opt/trn_rl_repo" not in sys.path:
    sys.path.insert(0, "/opt/trn_rl_repo")

import numpy as np

import concourse.bass as bass
import concourse.mybir as mybir
from concourse.bass_utils import run_bass_kernel_spmd

N_CORES = 8
B_FULL = 32
B_SHARD = B_FULL // N_CORES  # 4
J_IN = 31
J_OUT = 16
C = 64
T = 4096
P = 128
TT = (C * T) // P  # 2048
N_UNITS = B_SHARD * J_OUT  # 64 units per rep; unit k==0 is the root
NB_IN = 9  # tin slots (multiple of 3: slot sems ring-exclusive)
NB_SUM = 9  # tsum slots (multiple of 3; store sems ring-exclusive)
SLAG = 6  # store for unit j-6 issued at ring index j (3 | SLAG keeps ring)
N_RINGS = 3

f32 = mybir.dt.float32
bf16 = mybir.dt.bfloat16

_CACHE = {}


def _build_nc(reps: int = 1, out_dt=bf16) -> bass.Bass:
    nc = bass.Bass("TRN2", debug=False, num_devices=N_CORES)
    x = nc.dram_tensor("x", (B_SHARD, J_IN, C, T), f32, kind="ExternalInput")
    out = nc.dram_tensor("out", (B_SHARD, J_OUT, C, T), out_dt, kind="ExternalOutput")
    xp = x.ap().rearrange("b j c (u t) -> b (c u) j t", u=2)  # [4,128,31,2048]
    op = out.ap().rearrange("b j c (u t) -> b (c u) j t", u=2)  # [4,128,16,2048]

    tin = nc.alloc_sbuf_tensor("tin", [P, NB_IN * 2 * TT], f32)
    tsum = nc.alloc_sbuf_tensor("tsum", [P, NB_SUM * TT], out_dt)
    s_load = [nc.alloc_semaphore(f"s_load{i}") for i in range(NB_IN)]
    s_store = [nc.alloc_semaphore(f"s_store{i}") for i in range(NB_SUM)]
    s_add = nc.alloc_semaphore("s_add")
    s_mul = nc.alloc_semaphore("s_mul")

    TOT = reps * N_UNITS

    def task(g):
        b, k = divmod(g % N_UNITS, J_OUT)
        return b, k  # output joint k; inputs (2k-1, 2k), or joint 0 if k==0

    def tin_v(g, nj):
        s = (g % NB_IN) * 2 * TT
        return tin.ap()[:, s : s + nj * TT].rearrange("p (j t) -> p j t", j=nj)

    def tsum_v(g):
        s = (g % NB_SUM) * TT
        return tsum.ap()[:, s : s + TT]

    # The single sync-wait each DMA may carry is attached to the DMA itself
    # (not a standalone sequencer wait), so the ring sequencer never blocks
    # and keeps the DGE queue prefilled.

    def issue_load(eng, g):
        b, k = task(g)
        nj = 1 if k == 0 else 2
        j0 = 0 if k == 0 else 2 * k - 1
        inst = eng.dma_start(
            out=tin_v(g, nj), in_=xp[b, :, j0 : j0 + nj, :]
        ).then_inc(s_load[g % NB_IN], 16)
        if g >= NB_IN:
            # tin slot free once unit g-NB_IN's ADD has read it (the mul
            # only touches tsum, so gate on s_add, one op earlier)
            inst.wait_op(s_add, g - NB_IN + 1, "sem-ge")

    def issue_store(eng, g):
        b, k = task(g)
        eng.dma_start(out=op[b, :, k, :], in_=tsum_v(g)).then_inc(
            s_store[g % NB_SUM], 16
        ).wait_op(s_mul, g + 1, "sem-ge")

    def ring_prog(eng, r):
        for j in range(TOT + SLAG):
            if j < TOT and j % N_RINGS == r:
                issue_load(eng, j)
            js = j - SLAG
            if js >= 0 and j % N_RINGS == r:
                issue_store(eng, js)

    with nc.Block() as block:

        @block.sync
        def _(sync):
            ring_prog(sync, 0)
            # gate kernel end on all stores and roots; counts are exact
            # because slot reuse serializes same-sem DMAs.
            sync.wait_ge(s_mul, TOT)
            for s in range(NB_SUM):
                sync.wait_ge(s_store[s], 16 * len(range(s, TOT, NB_SUM)))

        @block.scalar
        def _(scalar):
            ring_prog(scalar, 1)

        @block.gpsimd
        def _(gpsimd):
            ring_prog(gpsimd, 2)

        @block.vector
        def _(vector):
            for g in range(TOT):
                b, k = task(g)
                vector.wait_ge(s_load[g % NB_IN], 16 * (g // NB_IN + 1))
                if g >= NB_SUM:
                    # tsum slot free once unit g-NB_SUM's store completed
                    vector.wait_ge(s_store[g % NB_SUM], 16 * (g // NB_SUM))
                nj = 1 if k == 0 else 2
                tv = tin_v(g, nj)
                sv = tsum_v(g)
                # DVE pipelines instructions (queue depth 8), so the RAW
                # add -> mul on the same tile needs an explicit sem edge
                vector.tensor_add(
                    out=sv, in0=tv[:, 0, :], in1=tv[:, nj - 1, :]
                ).then_inc(s_add, 1)
                vector.wait_ge(s_add, g + 1)
                # in-place halve: exact (power of two), same-AP in/out
                vector.tensor_scalar_mul(sv, sv, 0.5).then_inc(s_mul, 1)

    return nc


def get_nc() -> bass.Bass:
    if "nc" not in _CACHE:
        _CACHE["nc"] = _build_nc(1)
    return _CACHE["nc"]


def kernel(x: np.ndarray, **run_kwargs):
    x = np.ascontiguousarray(np.asarray(x, dtype=np.float32))
    assert x.shape == (B_FULL, J_IN, C, T), x.shape

    nc = get_nc()
    in_maps = [
        {"x": np.ascontiguousarray(x[i * B_SHARD : (i + 1) * B_SHARD])}
        for i in range(N_CORES)
    ]
    res = run_bass_kernel_spmd(nc, in_maps, core_ids=list(range(N_CORES)), **run_kwargs)
    out = np.concatenate(
        [np.asarray(res.results[i]["out"]) for i in range(N_CORES)], axis=0
    ).astype(np.float32)
    _CACHE["last_results"] = out if False else res
    return out


# revision 5
# speedup vs baseline: 1.2152x; 1.0110x over previous
"""SkeletalPool Trainium2 kernel, v4: three decoupled DMA rings, DVE-only
compute, bf16 stores, DRAM->DRAM root copies.

Computes out = (x[:, IDX0] + x[:, IDX1]) * 0.5 for the skeletal pooling
map: joint 0 passes through, joints (2i-1, 2i) average into output joint
i (i = 1..15).

  x:   [32, 31, 64, 4096] f32
  out: [32, 16, 64, 4096] bf16 on device, upcast to f32 on host.

Sharding: pure data parallelism over batch - 32 batches / 8 cores = 4
per core, no communication.

Measured hardware facts this design is built on (probe.py):
  - The HBM->SBUF load channel saturates at ~335 GB/s per core no matter
    how many DMA rings issue (1 ring: 333, 2: 342, 3: 301 GB/s). The
    kernel is load-bound: 120 MB of pair loads / ~335 GB/s ~ 358 us.
  - SBUF->HBM stores ride concurrently (baseline sustained 162 GB/s of
    writes on top of 314 GB/s reads), so bf16 stores (30 MB) are free.
  - DRAM->DRAM is slow (~123 GB/s) and interferes with loads, so only
    the 4 root-joint copies (pass-through, no compute) go that way -
    removing their 4 MB from the saturated load channel.
  - SWDGE (Pool) DMAs match HWDGE rings (~340 GB/s) and are the only
    path that can cast f32->bf16 in flight (used for the root copies).

Structure:
  - Work unit = 1 pair: load 2 input joints [128, 2, 2048] (8 KB
    descriptors, full 128-partition engagement - 16 KB descriptors with
    64 partitions measured SLOWER: 209 GB/s), DVE adds into a bf16 sum
    (one rounding, rel err <= 2^-8) and scales by 0.5 in place (exact,
    power of two), then the sum tile stores out. 60 units per rep.
  - Unit g's load and store both issue on ring g % 3 (SP / ACT / Pool);
    the store trails its ring by SLAG=3 indices. No compute runs on the
    ring engines, so rings never stall on the compute chain beyond the
    explicit slot-reuse gates.
  - Semaphore rules learned from the race detector / SWDGE ucode model:
    sems updated by SWDGE must be SWDGE-exclusive and hit exact
    cumulative targets; DMA +16 updates arrive as partial increments so
    a wait at 16n is exact only with one DMA in flight per sem. Slot
    counts are multiples of 3 so each slot sem is owned by exactly one
    ring, and slot reuse is gated through the compute chain (s_mul).
"""

import sys

if "/opt/trn_rl_repo" not in sys.path:
    sys.path.insert(0, "/opt/trn_rl_repo")

import numpy as np

import concourse.bass as bass
import concourse.mybir as mybir
from concourse.bass_utils import run_bass_kernel_spmd

N_CORES = 8
B_FULL = 32
B_SHARD = B_FULL // N_CORES  # 4
J_IN = 31
J_OUT = 16
C = 64
T = 4096
P = 128
TT = (C * T) // P  # 2048
N_PAIRS = 15
N_UNITS = B_SHARD * N_PAIRS  # 60 pair units per rep; roots go DRAM->DRAM
NB_IN = 9  # tin slots (multiple of 3: slot sems ring-exclusive)
NB_SUM = 9  # tsum slots (multiple of 3; store sems ring-exclusive)
SLAG = 3  # store for unit j-3 issued at ring index j (3 | SLAG keeps ring)
N_RINGS = 3

f32 = mybir.dt.float32
bf16 = mybir.dt.bfloat16

_CACHE = {}


def _build_nc(reps: int = 1, out_dt=bf16) -> bass.Bass:
    nc = bass.Bass("TRN2", debug=False, num_devices=N_CORES)
    x = nc.dram_tensor("x", (B_SHARD, J_IN, C, T), f32, kind="ExternalInput")
    out = nc.dram_tensor("out", (B_SHARD, J_OUT, C, T), out_dt, kind="ExternalOutput")
    xp = x.ap().rearrange("b j c (u t) -> b (c u) j t", u=2)  # [4,128,31,2048]
    op = out.ap().rearrange("b j c (u t) -> b (c u) j t", u=2)  # [4,128,16,2048]

    tin = nc.alloc_sbuf_tensor("tin", [P, NB_IN * 2 * TT], f32)
    tsum = nc.alloc_sbuf_tensor("tsum", [P, NB_SUM * TT], out_dt)
    s_load = [nc.alloc_semaphore(f"s_load{i}") for i in range(NB_IN)]
    s_store = [nc.alloc_semaphore(f"s_store{i}") for i in range(NB_SUM)]
    s_root = [nc.alloc_semaphore(f"s_root{i}") for i in range(2)]
    s_add = nc.alloc_semaphore("s_add")
    s_mul = nc.alloc_semaphore("s_mul")

    TOT = reps * N_UNITS
    N_ROOTS = reps * B_SHARD

    def task(g):
        b, k1 = divmod(g % N_UNITS, N_PAIRS)
        return b, k1 + 1  # output joint k; inputs (2k-1, 2k)

    def tin_v(g):
        s = (g % NB_IN) * 2 * TT
        return tin.ap()[:, s : s + 2 * TT].rearrange("p (j t) -> p j t", j=2)

    def tsum_v(g):
        s = (g % NB_SUM) * TT
        return tsum.ap()[:, s : s + TT]

    def issue_load(eng, g):
        b, k = task(g)
        if g >= NB_IN:
            # tin slot free once unit g-NB_IN's add has read it
            eng.wait_ge(s_mul, g - NB_IN + 1)
        j0 = 2 * k - 1
        eng.dma_start(out=tin_v(g), in_=xp[b, :, j0 : j0 + 2, :]).then_inc(
            s_load[g % NB_IN], 16
        )

    def issue_store(eng, g):
        b, k = task(g)
        eng.wait_ge(s_mul, g + 1)
        eng.dma_start(out=op[b, :, k, :], in_=tsum_v(g)).then_inc(
            s_store[g % NB_SUM], 16
        )

    def issue_root(eng, ri):
        # root joint passes through exactly: DRAM->DRAM cast copy (SWDGE),
        # riding the D2D path instead of the saturated HBM->SBUF channel.
        b = ri % B_SHARD
        if ri >= 2:
            eng.wait_ge(s_root[ri % 2], 16 * (ri // 2))
        eng.dma_start(out=op[b, :, 0, :], in_=xp[b, :, 0, :]).then_inc(
            s_root[ri % 2], 16
        )

    # fire root (rep, b) at the first Pool ring index inside that batch's
    # unit range
    root_at = {}
    for ri in range(N_ROOTS):
        j0 = ri * N_PAIRS
        root_at[j0 + ((2 - j0) % N_RINGS)] = ri

    def ring_prog(eng, r):
        for j in range(TOT + SLAG):
            if r == 2 and j in root_at:
                issue_root(eng, root_at[j])
            if j < TOT and j % N_RINGS == r:
                issue_load(eng, j)
            js = j - SLAG
            if js >= 0 and j % N_RINGS == r:
                issue_store(eng, js)

    with nc.Block() as block:

        @block.sync
        def _(sync):
            ring_prog(sync, 0)
            # gate kernel end on all stores and roots; counts are exact
            # because slot reuse serializes same-sem DMAs.
            sync.wait_ge(s_mul, TOT)
            for s in range(NB_SUM):
                sync.wait_ge(s_store[s], 16 * len(range(s, TOT, NB_SUM)))
            for i in range(2):
                sync.wait_ge(s_root[i], 16 * len(range(i, N_ROOTS, 2)))

        @block.scalar
        def _(scalar):
            ring_prog(scalar, 1)

        @block.gpsimd
        def _(gpsimd):
            ring_prog(gpsimd, 2)

        @block.vector
        def _(vector):
            for g in range(TOT):
                vector.wait_ge(s_load[g % NB_IN], 16 * (g // NB_IN + 1))
                if g >= NB_SUM:
                    # tsum slot free once unit g-NB_SUM's store completed
                    vector.wait_ge(s_store[g % NB_SUM], 16 * (g // NB_SUM))
                tv = tin_v(g)
                sv = tsum_v(g)
                # DVE pipelines instructions (queue depth 8), so the RAW
                # add -> mul on the same tile needs an explicit sem edge
                vector.tensor_add(out=sv, in0=tv[:, 0, :], in1=tv[:, 1, :]).then_inc(
                    s_add, 1
                )
                vector.wait_ge(s_add, g + 1)
                # in-place halve: exact (power of two), same-AP in/out
                vector.tensor_scalar_mul(sv, sv, 0.5).then_inc(s_mul, 1)

    return nc


def get_nc() -> bass.Bass:
    if "nc" not in _CACHE:
        _CACHE["nc"] = _build_nc(1)
    return _CACHE["nc"]


def kernel(x: np.ndarray, **run_kwargs):
    x = np.ascontiguousarray(np.asarray(x, dtype=np.float32))
    assert x.shape == (B_FULL, J_IN, C, T), x.shape

    nc = get_nc()
    in_maps = [
        {"x": np.ascontiguousarray(x[i * B_SHARD : (i + 1) * B_SHARD])}
        for i in range(N_CORES)
    ]
    res = run_bass_kernel_spmd(nc, in_maps, core_ids=list(range(N_CORES)), **run_kwargs)
    out = np.concatenate(
        [np.asarray(res.results[i]["out"]) for i in range(N_CORES)], axis=0
    ).astype(np.float32)
    _CACHE["last_results"] = out if False else res
    return out
